# revision 2
# baseline (speedup 1.0000x reference)
"""v3: 5x5 conv2d with FULL PE array usage via 2x column tiling.

Baseline used 4 row-tiled 32x64 matmuls (half the PE columns idle).
v3 runs 8 concurrent tiles (4 row groups x 2 col groups): col group 0
computes output rows [y, y+4), col group 1 rows [y+4, y+8) of the same
image, sharing the per-tap weights. PSUM tile [128, 4, 128]: partitions
0-63 = col-0 output block, 64-127 = col-64 block.

SCHEME "A": 1 MM per tile per tap, 4 live banks/round (clean eviction
overlap via pool rotation). SCHEME "B": 2 MMs per tile per tap (LDW
amortized), 8 live banks, eviction stalls round boundary.
DT: "bf16" or "f32r".
"""

import os
import numpy as np
import ml_dtypes

import concourse.bass as bass
import concourse.tile as tile
from concourse import bacc, mybir
from concourse.bass_utils import run_bass_kernel_spmd

N_CORES = 8
IMGS = 4
C = 32
O = 64
H = W = 128
KH = KW = 5
PAD = 2
WP = W + 2 * PAD      # 132
BANDS = 4
BAND_OUT = H // BANDS  # 32
BAND_IN = BAND_OUT + 2 * PAD  # 36
TAPS = KH * KW
RT = 4                 # rows per half-psum-tile

F32 = mybir.dt.float32
BF16 = mybir.dt.bfloat16
F32R = mybir.dt.float32r

SCHEME = "A"
DT = "bf16"


def _build_nc(reps=1):
    mm_dt = BF16 if DT == "bf16" else F32R
    io_dt = BF16 if DT == "bf16" else F32

    def cast(ap):
        return ap if DT == "bf16" else ap.bitcast(F32R)

    nc = bacc.Bacc("TRN2", target_bir_lowering=False, debug=False)
    X = nc.dram_tensor("X", [IMGS, C, H, W], io_dt, kind="ExternalInput").ap()
    K = nc.dram_tensor("K", [O, C, KH, KW], io_dt, kind="ExternalInput").ap()
    Z = nc.dram_tensor("Z", [128, BAND_IN, 2 * PAD], io_dt, kind="ExternalInput").ap()
    ZR = nc.dram_tensor("ZR", [128, PAD, WP], io_dt, kind="ExternalInput").ap()
    out = nc.dram_tensor("out", [IMGS, O, H, W], F32, kind="ExternalOutput").ap()

    taps = [(dy, dx) for dy in range(KH) for dx in range(KW)]

    with tile.TileContext(nc) as tc:
        with (
            tc.tile_pool(name="wpool", bufs=1) as wpool,
            tc.tile_pool(name="xpool", bufs=3) as xpool,
            tc.tile_pool(name="opool", bufs=8) as opool,
            tc.tile_pool(name="ppool", bufs=8, space="PSUM") as ppool,
        ):
            wt = wpool.tile([128, TAPS, O], mm_dt)
            ksrc = cast(K.rearrange("o c h w -> c (h w) o"))
            for g in range(IMGS):
                nc.sync.dma_start(wt[32 * g : 32 * g + 32, :, :], ksrc)

            def load_band(b):
                y0 = b * BAND_OUT
                xb = xpool.tile([128, BAND_IN, WP], mm_dt)
                p_lo = PAD if b == 0 else 0
                p_hi = BAND_IN - 1 - PAD if b == BANDS - 1 else BAND_IN - 1
                r_lo = y0 + p_lo - PAD
                r_hi = y0 + p_hi - PAD
                nc.sync.dma_start(xb[:, :, 0:PAD], cast(Z[:, :, 0:PAD]))
                nc.sync.dma_start(
                    xb[:, :, PAD + W : WP], cast(Z[:, :, PAD : 2 * PAD])
                )
                if b == 0:
                    nc.sync.dma_start(xb[:, 0:PAD, :], cast(ZR))
                if b == BANDS - 1:
                    nc.sync.dma_start(
                        xb[:, BAND_IN - PAD : BAND_IN, :], cast(ZR)
                    )
                for g in range(IMGS):
                    nc.sync.dma_start(
                        xb[32 * g : 32 * g + 32, p_lo : p_hi + 1, PAD : PAD + W],
                        cast(X[g, :, r_lo : r_hi + 1, :]),
                    )
                return xb

            def evict(ps_g, y0):
                # ps_g: list of 4 [128,4,128] tiles, one per image;
                # partitions 0:64 = rows y0..y0+3, 64:128 = y0+4..y0+7
                for g in range(IMGS):
                    ob = opool.tile([128, RT, W], F32, name="ob", tag="ob")
                    eng = nc.vector if g % 2 == 0 else nc.scalar
                    if g % 2 == 0:
                        eng.tensor_copy(ob[:, :, :], ps_g[g][:, :, :])
                    else:
                        eng.activation(
                            ob[:, :, :], ps_g[g][:, :, :],
                            mybir.ActivationFunctionType.Copy,
                        )
                    nc.sync.dma_start(out[g, :, y0 : y0 + RT, :], ob[0:O])
                    nc.sync.dma_start(
                        out[g, :, y0 + RT : y0 + 2 * RT, :], ob[O:128]
                    )

            def evict64(ps_g, y0):
                # col-0-only variant: ps_g are [64,4,128] tiles
                for g in range(IMGS):
                    ob = opool.tile([O, RT, W], F32, name="ob64", tag="ob64")
                    if g % 2 == 0:
                        nc.vector.tensor_copy(ob[:, :, :], ps_g[g][:, :, :])
                    else:
                        nc.scalar.activation(
                            ob[:, :, :], ps_g[g][:, :, :],
                            mybir.ActivationFunctionType.Copy,
                        )
                    nc.sync.dma_start(out[g, :, y0 : y0 + RT, :], ob)

            def body():
                for b in range(BANDS):
                    xb = load_band(b)
                    if SCHEME == "C":
                        # baseline structure, bf16: col 0 only, 1 MM/tile/tap
                        for t in range(BAND_OUT // RT):  # 8 rounds
                            ps_g = [
                                ppool.tile([O, RT, W], F32, name=f"c_{b}_{t}_{g}", tag="ps")
                                for g in range(IMGS)
                            ]
                            ybase = RT * t
                            for ti, (dy, dx) in enumerate(taps):
                                for g in range(IMGS):
                                    nc.tensor.matmul(
                                        ps_g[g][:, :, :],
                                        wt[32 * g : 32 * g + 32, dy * KW + dx, :],
                                        xb[
                                            32 * g : 32 * g + 32,
                                            ybase + dy : ybase + dy + RT,
                                            dx : dx + W,
                                        ],
                                        start=ti == 0, stop=ti == TAPS - 1,
                                        tile_position=(32 * g, 0),
                                    )
                            evict64(ps_g, b * BAND_OUT + RT * t)
                        continue
                    if SCHEME == "D":
                        # col 0 only, 2 MMs per tile per tap (LDW amortized)
                        for t in range(BAND_OUT // (2 * RT)):  # 4 rounds
                            p1 = [
                                ppool.tile([O, RT, W], F32, name=f"d1_{b}_{t}_{g}", tag="ps")
                                for g in range(IMGS)
                            ]
                            p2 = [
                                ppool.tile([O, RT, W], F32, name=f"d2_{b}_{t}_{g}", tag="ps")
                                for g in range(IMGS)
                            ]
                            base = 2 * RT * t
                            for ti, (dy, dx) in enumerate(taps):
                                for g in range(IMGS):
                                    lhsT = wt[32 * g : 32 * g + 32, dy * KW + dx, :]
                                    for pj, off in ((p1, 0), (p2, RT)):
                                        nc.tensor.matmul(
                                            pj[g][:, :, :],
                                            lhsT,
                                            xb[
                                                32 * g : 32 * g + 32,
                                                base + off + dy : base + off + dy + RT,
                                                dx : dx + W,
                                            ],
                                            start=ti == 0, stop=ti == TAPS - 1,
                                            tile_position=(32 * g, 0),
                                        )
                            evict64(p1, b * BAND_OUT + base)
                            evict64(p2, b * BAND_OUT + base + RT)
                        continue
                    if SCHEME == "A":
                        for t in range(BAND_OUT // (2 * RT)):  # 4 rounds
                            ps_g = [
                                ppool.tile([128, RT, W], F32, name=f"ps_{b}_{t}_{g}", tag="ps")
                                for g in range(IMGS)
                            ]
                            top = 2 * RT * t
                            bot = top + RT
                            for ti, (dy, dx) in enumerate(taps):
                                first = ti == 0
                                last = ti == TAPS - 1
                                for g in range(IMGS):
                                    lhsT = wt[32 * g : 32 * g + 32, dy * KW + dx, :]
                                    rhs_t = xb[
                                        32 * g : 32 * g + 32,
                                        top + dy : top + dy + RT,
                                        dx : dx + W,
                                    ]
                                    rhs_b = xb[
                                        32 * g : 32 * g + 32,
                                        bot + dy : bot + dy + RT,
                                        dx : dx + W,
                                    ]
                                    nc.tensor.matmul(
                                        ps_g[g][0:O, :, :], lhsT, rhs_t,
                                        start=first, stop=last,
                                        tile_position=(32 * g, 0),
                                    )
                                    nc.tensor.matmul(
                                        ps_g[g][O:128, :, :], lhsT, rhs_b,
                                        start=first, stop=last,
                                        tile_position=(32 * g, O),
                                    )
                            evict(ps_g, b * BAND_OUT + 2 * RT * t)
                    else:  # SCHEME B: 2 rounds of 16 rows, LDW amortized 2x
                        for t in range(BAND_OUT // (4 * RT)):  # 2 rounds
                            ps1 = [
                                ppool.tile([128, RT, W], F32, name=f"ps_{b}_{t}_{g}", tag="ps")
                                for g in range(IMGS)
                            ]
                            ps2 = [
                                ppool.tile([128, RT, W], F32, name=f"ps_{b}_{t}_{g}", tag="ps")
                                for g in range(IMGS)
                            ]
                            base = 4 * RT * t
                            for ti, (dy, dx) in enumerate(taps):
                                first = ti == 0
                                last = ti == TAPS - 1
                                for g in range(IMGS):
                                    lhsT = wt[32 * g : 32 * g + 32, dy * KW + dx, :]

                                    def rhs(off):
                                        return xb[
                                            32 * g : 32 * g + 32,
                                            base + off + dy : base + off + dy + RT,
                                            dx : dx + W,
                                        ]

                                    nc.tensor.matmul(
                                        ps1[g][0:O, :, :], lhsT, rhs(0),
                                        start=first, stop=last,
                                        tile_position=(32 * g, 0),
                                    )
                                    nc.tensor.matmul(
                                        ps2[g][0:O, :, :], lhsT, rhs(2 * RT),
                                        start=first, stop=last,
                                        tile_position=(32 * g, 0),
                                    )
                                    nc.tensor.matmul(
                                        ps1[g][O:128, :, :], lhsT, rhs(RT),
                                        start=first, stop=last,
                                        tile_position=(32 * g, O),
                                    )
                                    nc.tensor.matmul(
                                        ps2[g][O:128, :, :], lhsT, rhs(3 * RT),
                                        start=first, stop=last,
                                        tile_position=(32 * g, O),
                                    )
                            evict(ps1, b * BAND_OUT + 4 * RT * t)
                            evict(ps2, b * BAND_OUT + 4 * RT * t + 2 * RT)

            if reps > 1:
                with tc.For_i(0, reps, 1):
                    body()
            else:
                body()
    nc.compile()
    return nc


_CACHE = {}


def _get_nc(reps=1):
    if reps not in _CACHE:
        _CACHE[reps] = _build_nc(reps)
    return _CACHE[reps]


def _io_np_dtype():
    return ml_dtypes.bfloat16 if DT == "bf16" else np.float32


def make_in_maps(X, K):
    dt = _io_np_dtype()
    X = np.ascontiguousarray(np.asarray(X), dtype=np.float32)
    K = np.ascontiguousarray(np.asarray(K), dtype=np.float32)
    per = X.shape[0] // N_CORES
    Z = np.zeros((128, BAND_IN, 2 * PAD), dtype=dt)
    ZR = np.zeros((128, PAD, WP), dtype=dt)
    Kc = K.astype(dt)
    return [
        {
            "X": np.ascontiguousarray(X[per * i : per * (i + 1)]).astype(dt),
            "K": Kc,
            "Z": Z,
            "ZR": ZR,
        }
        for i in range(N_CORES)
    ]


def kernel(X, K):
    nc = _get_nc()
    in_maps = make_in_maps(X, K)
    res = run_bass_kernel_spmd(nc, in_maps, list(range(N_CORES))).results
    return np.concatenate([res[i]["out"] for i in range(N_CORES)], axis=0)
